# revision 48
# baseline (speedup 1.0000x reference)
"""TRN2 Bass kernel for nn_NMS (offset min-sum LDPC decoder, batch 256).

Self-contained: derives all index tables from the H input at call time,
shards the batch across 8 NeuronCores (32 per core), runs one SPMD Bass
program via run_bass_kernel_spmd, and gathers the full [256, 576] output.

Per-core layout: two independent 16-batch streams, each using all 128
partitions as 8 row-blocks x 16 batch; a stream's edges live on the free
axis as [18 rows x 16 slots] = 288 fp16 values.  The two streams pipeline
against each other (stream B's row-stat DVE work overlaps stream A's
gather/matmul colsum phase).  All edge tensors are fp16; fp16 ties are
broken by truncating |X| to 6 mantissa bits and embedding the slot index
k in the cleared LSBs, which reproduces the reference's second-smallest-
with-multiplicity semantics.

Per iteration, per stream:
  X   = gather(zrep) - E          (Pool indirect_copy + DVE sub; iter 0
                                   uses a host-precomputed X0 DMA)
  tb  = (X & 0x7FF0) | ktab       (DVE u16 bit ops)
  m1  = segmin(tb); mask = (tb == m1); m2 = segmin(tb + 30000*mask)
  sgn = (X & 0x8000) | 0x3C00     (exact +-1, never 0)
  p   = segprod(sgn)              (DVE mult-reduce, exact +-1 in fp32)
  w12 = p * Relu(alpha*m12 - alpha*beta)            (ACT relu, DVE mul)
  E   = sgn * select(mask, w2, w1)                  (DVE copy_predicated)
  colsum: two 576-wide strip gathers from [E | 0 | r | aux] feed two
  accumulating fp16 PE matmuls per column-half (one-hot cross-block
  sum + replicate); each column's r enters via a strip slot pointing
  into a per-block r region, so zps = colsum + r directly.  Columns
  whose strips overflow use aux slots pre-summed by tiny DVE adds.
  ACT evacuates PSUM to fp16 zrep (iters 0-1) / fp32 zout (final).
"""
import numpy as np
from contextlib import ExitStack

import concourse.bass as bass
import concourse.tile as tile
from concourse import mybir

FP32 = mybir.dt.float32
FP16 = mybir.dt.float16
I32 = mybir.dt.int32
U16 = mybir.dt.uint16
ALU = mybir.AluOpType
AX = mybir.AxisListType
AF = mybir.ActivationFunctionType

P = 128
NSTR = 2         # independent batch streams per core
B = 16           # batch per stream
BC = NSTR * B    # batch per core
NBLK = 8
RPB = 18         # rows per block
KPAD = 16        # padded row degree
ROW_DEG = 15
EPB = RPB * KPAD  # 288 edge slots per block per stream
N = 576          # columns
HALF = N // 2
D_KEEP = 2       # strip slots per (block, col)
ITERS = 3
BIG = np.float16(30000.0)
ZSLOT = EPB           # zero slot in E_ext
RBASE = EPB + 1       # per-block r region base
RW = 104              # per-block r region width
AUXB = RBASE + RW     # aux slots base
NAUX = 32
EXTW = AUXB + NAUX    # gather window width


# ---------------------------------------------------------------- tables ----
def build_tables(H):
    MROWS = H.shape[0]
    cols = np.array([np.nonzero(H[m])[0] for m in range(MROWS)], dtype=np.int64)
    assert cols.shape == (MROWS, ROW_DEG)
    coldeg = H.sum(0)
    heat = np.array([-coldeg[cols[m]].max() for m in range(MROWS)])
    order = list(np.argsort(heat, kind="stable"))
    blocks = [[] for _ in range(NBLK)]
    cnt = np.zeros((NBLK, N), dtype=np.int32)
    for m in order:
        best, bestkey = None, None
        for j in range(NBLK):
            if len(blocks[j]) >= RPB:
                continue
            key = tuple(np.sort(cnt[j, cols[m]])[::-1])
            if best is None or key < bestkey:
                best, bestkey = j, key
        blocks[best].append(m)
        cnt[best, cols[m]] += 1
    assign = np.zeros(MROWS, dtype=np.int64)
    for j, b in enumerate(blocks):
        for m in b:
            assign[m] = j

    def cost_vec(c):
        h = np.bincount(c.flatten(), minlength=8)
        return (int(h[4:].sum()), int(h[3]), int(h[2]))

    for _ in range(12):
        improved = False
        for m1 in range(MROWS):
            for m2 in range(m1 + 1, MROWS):
                j1, j2 = assign[m1], assign[m2]
                if j1 == j2:
                    continue
                c = cnt.copy()
                c[j1, cols[m1]] -= 1
                c[j1, cols[m2]] += 1
                c[j2, cols[m2]] -= 1
                c[j2, cols[m1]] += 1
                if cost_vec(c) < cost_vec(cnt):
                    assign[m1], assign[m2] = j2, j1
                    cnt = c
                    improved = True
        if not improved or cnt.max() <= D_KEEP + 1:
            break
    assert cnt.max() <= D_KEEP + 1, f"strip depth {cnt.max()} > {D_KEEP + 1}"
    rows_of_block = [np.array([m for m in range(MROWS) if assign[m] == j],
                              dtype=np.int64) for j in range(NBLK)]

    colidx = np.full((NBLK, RPB, KPAD), N, dtype=np.int64)
    for j in range(NBLK):
        for mm, m in enumerate(rows_of_block[j]):
            colidx[j, mm, :ROW_DEG] = cols[m]

    pos_lists = [[[] for _ in range(N)] for _ in range(NBLK)]
    for j in range(NBLK):
        for mm in range(RPB):
            for k in range(ROW_DEG):
                n = colidx[j, mm, k]
                pos_lists[j][n].append(mm * KPAD + k)

    # r-slot assignment: each col's r lives in one block's strips (compact
    # per-block r region keeps the gather window small)
    r_block = np.full(N, -1, dtype=np.int64)
    rfull = []
    rcols = [[] for _ in range(NBLK)]
    r_rank = np.full(N, -1, dtype=np.int64)
    for n in range(N):
        cand = [j for j in range(NBLK) if len(pos_lists[j][n]) < D_KEEP]
        if cand:
            j = min(cand, key=lambda j: (len(rcols[j]), len(pos_lists[j][n])))
            r_block[n] = j
            r_rank[n] = len(rcols[j])
            rcols[j].append(int(n))
        else:
            rfull.append(int(n))
    assert len(rfull) <= 16, f"too many rfull cols: {len(rfull)}"

    # strip slots; overflow entries become aux slots (pre-summed pairs)
    slots = np.full((NBLK, N, D_KEEP), ZSLOT, dtype=np.int64)
    aux = []
    for j in range(NBLK):
        for n in range(N):
            lst = list(pos_lists[j][n])
            if r_block[n] == j:
                lst.append(RBASE + int(r_rank[n]))
            assert len(lst) <= D_KEEP + 1, f"depth {len(lst)} at {(j, n)}"
            if len(lst) > D_KEEP:
                q, extra = lst[D_KEEP - 1], lst[D_KEEP]
                lst = lst[:D_KEEP - 1] + [AUXB + len(aux)]
                aux.append((int(q), int(extra)))
            for d, v in enumerate(lst):
                slots[j, n, d] = v
    for n in rfull:
        for j in range(NBLK):
            if slots[j, n, D_KEEP - 1] < EPB and len(rcols[j]) < RW:
                q = int(slots[j, n, D_KEEP - 1])
                slots[j, n, D_KEEP - 1] = AUXB + len(aux)
                aux.append((q, RBASE + len(rcols[j])))
                rcols[j].append(int(n))
                break
        else:
            raise AssertionError(f"no aux-able slot for rfull col {n}")
    assert len(aux) <= NAUX, f"too many aux slots: {len(aux)}"
    assert max(len(c) for c in rcols) <= RW, \
        f"r region overflow: {[len(c) for c in rcols]}"

    # ---- wrapped uint16 index tensors (one 16-partition group per block) ----
    def wrap(vals_per_block, num_idxs):
        t = np.zeros((P, num_idxs // 16), dtype=np.uint16)
        for c in range(8):
            v = vals_per_block[c]
            for i in range(num_idxs):
                t[16 * c + i % 16, i // 16] = v[i]
        return t

    zvals = [colidx[j].reshape(-1) for j in range(NBLK)]           # 288 each
    svals = []
    for j in range(NBLK):
        v = np.empty(N * D_KEEP, dtype=np.int64)
        i = 0
        for h in range(2):
            for d in range(D_KEEP):
                for c in range(HALF):
                    v[i] = slots[j, h * HALF + c, d]
                    i += 1
        svals.append(v)
    zidx = wrap(zvals, EPB)
    sidx = wrap(svals, N * D_KEEP)

    # one-hot cross-block sum + replicate: W[(j',b'), (j,b)] = (b'==b)
    wmat = np.zeros((P, P), dtype=np.float16)
    for jp in range(NBLK):
        for bp in range(B):
            for j in range(NBLK):
                wmat[jp * B + bp, j * B + bp] = 1.0
    ktab = np.tile(np.arange(KPAD, dtype=np.uint16), (P, RPB))
    return dict(zidx=zidx, sidx=sidx, wmat=wmat, colidx=colidx, ktab=ktab,
                aux=aux, rcols=rcols)


def build_x0(r_stream, colidx):
    """Iteration-0 gather: x0[(j,b), (mm,k)] = r[b, col] (pads BIG)."""
    rh = r_stream.astype(np.float16)
    rpad = np.concatenate([rh, np.full((B, 1), BIG, np.float16)], axis=1)
    x0 = rpad[:, colidx]                      # [B, NBLK, RPB, KPAD]
    x0 = x0.transpose(1, 0, 2, 3).reshape(P, EPB)
    return np.ascontiguousarray(x0)


def build_r4(r_stream, rcols):
    """Per-block r region of E_ext: out[(j,b), i] = r[b, rcols[j][i]]."""
    rh = r_stream.astype(np.float16)
    out = np.zeros((P, RW), np.float16)
    for j in range(NBLK):
        cs = rcols[j]
        out[j * B:(j + 1) * B, 0:len(cs)] = rh[:, cs]
    return np.ascontiguousarray(out)


# ---------------------------------------------------------------- kernel ----
def hoist_waits(nc, max_embedded=1):
    """Split multi-wait instructions into standalone EventSemaphore waits.

    The walrus build used by the axon compile path only supports a single
    sync-wait slot on most TPB instruction structs; Tile attaches one wait
    per producer proc.  Hoist the extras onto the instruction's sequencer
    as separate wait instructions (exactly what raw-bass wait_ge emits).
    """
    k = 0
    for f in nc.m.functions:
        for b in f.blocks:
            insts = b.instructions
            out = []
            for i in insts:
                tname = type(i).__name__
                si = i.sync_info
                if (si is not None and tname != "InstEventSemaphore"
                        and len(si.on_wait) > max_embedded):
                    waits = list(si.on_wait)
                    keep = waits[:max_embedded]
                    for w in waits[max_embedded:]:
                        es = mybir.InstEventSemaphore(
                            name=f"hoistw{k}", ins=[], outs=[])
                        k += 1
                        es.engine = i.engine
                        es.sync_info = mybir.SyncInfo(on_wait=[w], on_update=[])
                        nc.inst_map[es.name] = es
                        out.append(es)
                    i.sync_info = mybir.SyncInfo(
                        on_wait=keep, on_update=list(si.on_update))
                out.append(i)
            b.instructions = out


def build_bass(alpha, beta, aux=()):
    """alpha/beta: lists of 3 floats (baked as immediates).
    aux: [(q, extra)] E_ext position pairs pre-summed into aux slots."""
    nc = bass.Bass("TRN2", target_bir_lowering=False, debug=False)
    x0_d = [nc.dram_tensor(f"x0_{s}", [P, EPB], FP16, kind="ExternalInput")
            for s in range(NSTR)]
    r4_d = [nc.dram_tensor(f"r4_{s}", [P, RW], FP16, kind="ExternalInput")
            for s in range(NSTR)]
    zidx_d = nc.dram_tensor("zidx", [P, EPB // 16], U16, kind="ExternalInput")
    sidx_d = nc.dram_tensor("sidx", [P, N * D_KEEP // 16], U16, kind="ExternalInput")
    ktab_d = nc.dram_tensor("ktab", [P, EPB], U16, kind="ExternalInput")
    wmat_d = nc.dram_tensor("wmat", [P, P], FP16, kind="ExternalInput")
    out_d = nc.dram_tensor("out", [BC, N], FP32, kind="ExternalOutput")

    with tile.TileContext(nc) as tc:
        with ExitStack() as ctx:
            pool = ctx.enter_context(tc.tile_pool(name="main", bufs=1))
            pspool = ctx.enter_context(tc.tile_pool(name="ps", bufs=1, space="PSUM"))

            def stile(shape, dtype, name):
                return [pool.tile(shape, dtype, name=f"{name}{s}")
                        for s in range(NSTR)]

            E = stile([P, EXTW + 1], FP16, "E")
            zrep = stile([P, N + 2], FP16, "zrep")
            Xg = stile([P, EPB], FP16, "Xg")
            X = stile([P, RPB, KPAD], FP16, "X")
            tb = stile([P, RPB, KPAD], FP16, "tb")
            mask = stile([P, RPB, KPAD], FP16, "mask")
            tB = stile([P, RPB, KPAD], FP16, "tB")
            masked = stile([P, RPB, KPAD], FP16, "masked")
            sgn = stile([P, RPB, KPAD], FP16, "sgn")
            w2m = stile([P, RPB, KPAD], FP16, "w2m")
            m12 = stile([P, 2, RPB], FP16, "m12")
            prod = stile([P, RPB], FP32, "prod")
            w12 = stile([P, 2, RPB], FP16, "w12")
            G = stile([P, 2, D_KEEP, HALF], FP16, "G")
            zout = stile([P, N], FP32, "zout")
            zpsA = [pspool.tile([P, HALF], FP32, name=f"zpsA{s}")
                    for s in range(NSTR)]
            zpsB = [pspool.tile([P, HALF], FP32, name=f"zpsB{s}")
                    for s in range(NSTR)]
            zidx = pool.tile([P, EPB // 16], U16)
            sidx = pool.tile([P, N * D_KEEP // 16], U16)
            ktab = pool.tile([P, RPB, KPAD], U16)
            wmat = pool.tile([P, P], FP16)
            biasw = [pool.tile([P, 1], FP32, name=f"biasw{i}")
                     for i in range(ITERS)]

            # ---- static loads ----
            nc.scalar.dma_start(wmat[:], wmat_d[:])
            nc.gpsimd.dma_start(zidx[:], zidx_d[:])
            nc.gpsimd.dma_start(sidx[:], sidx_d[:])
            nc.gpsimd.dma_start(ktab[:].rearrange("p a b -> p (a b)"), ktab_d[:])
            for s in range(NSTR):
                nc.gpsimd.dma_start(E[s][:, RBASE:RBASE + RW], r4_d[s][:])
                nc.vector.memset(E[s][:, ZSLOT:ZSLOT + 1], 0.0)
                nc.vector.memset(zrep[s][:, N:N + 2], float(BIG))
            for it in range(ITERS):
                nc.vector.memset(biasw[it][:], -float(alpha[it]) * float(beta[it]))
            idxtouch = pool.tile([P, 2], U16)
            nc.gpsimd.tensor_copy(idxtouch[:, 0:1], zidx[:, 0:1])
            nc.tensor.matmul(zpsA[0][0:1, 0:1], lhsT=wmat[0:B, 0:1],
                             rhs=wmat[0:B, 0:1], start=True, stop=True)

            for it in range(ITERS):
                al = float(alpha[it])
                for s in range(NSTR):
                    Xf = X[s][:].rearrange("p a b -> p (a b)")
                    Xu = X[s][:].bitcast(U16)
                    tbu = tb[s][:].bitcast(U16)
                    sgu = sgn[s][:].bitcast(U16)
                    Eedge = E[s][:, 0:EPB].rearrange("p (a b) -> p a b", a=RPB)

                    # ---- X = gather(zrep) - E ----
                    # high_priority: the scheduler must prefer this gather
                    # over the other stream's strip gathers, else the DVE
                    # starves waiting for X at each iteration boundary
                    if it == 0:
                        nc.sync.dma_start(Xf, x0_d[s][:])
                    else:
                        with tc.high_priority(offset=40):
                            nc.gpsimd.indirect_copy(Xg[s][:],
                                                    zrep[s][:, 0:N + 1],
                                                    zidx[:], True)
                        nc.vector.tensor_tensor(Xf, Xg[s][:],
                                                E[s][:, 0:EPB],
                                                op=ALU.subtract)

                    # ---- tie-broken |X| and mask/min chain ----
                    nc.vector.tensor_single_scalar(tbu, Xu, 0x7FF0,
                                                   op=ALU.bitwise_and)
                    nc.vector.tensor_tensor(tbu, tbu, ktab[:], op=ALU.bitwise_or)
                    nc.vector.tensor_reduce(m12[s][:, 0, :], tb[s][:],
                                            axis=AX.X, op=ALU.min)
                    m1b = m12[s][:, 0, :].unsqueeze(2).broadcast_to(
                        [P, RPB, KPAD])
                    nc.vector.tensor_tensor(mask[s][:], tb[s][:], m1b,
                                            op=ALU.is_equal)
                    nc.vector.tensor_scalar(tB[s][:], mask[s][:], float(BIG),
                                            0.0, op0=ALU.mult, op1=ALU.add)
                    nc.vector.tensor_tensor(masked[s][:], tb[s][:], tB[s][:],
                                            op=ALU.add)
                    nc.vector.tensor_reduce(m12[s][:, 1, :], masked[s][:],
                                            axis=AX.X, op=ALU.min)
                    nc.scalar.activation(w12[s][:], m12[s][:], func=AF.Relu,
                                         scale=al, bias=biasw[it][:])

                    # ---- parity: sgn = (X&0x8000)|0x3C00; p = prod(sgn) ----
                    nc.vector.tensor_single_scalar(sgu, Xu, 0x8000,
                                                   op=ALU.bitwise_and)
                    nc.vector.tensor_single_scalar(sgu, sgu, 0x3C00,
                                                   op=ALU.bitwise_or)
                    nc.vector.tensor_reduce(prod[s][:], sgn[s][:], axis=AX.X,
                                            op=ALU.mult)
                    prb = prod[s][:].unsqueeze(1).broadcast_to([P, 2, RPB])
                    nc.vector.tensor_tensor(w12[s][:], w12[s][:], prb,
                                            op=ALU.mult)

                    # ---- E = sgn * select(mask, w2, w1) ----
                    w1b = w12[s][:, 0, :].unsqueeze(2).broadcast_to(
                        [P, RPB, KPAD])
                    w2b = w12[s][:, 1, :].unsqueeze(2).broadcast_to(
                        [P, RPB, KPAD])
                    nc.vector.tensor_copy(w2m[s][:], w2b)
                    nc.scalar.activation(Eedge, w1b, func=AF.Copy)
                    masku = mask[s][:].bitcast(U16).rearrange("p a b -> p (a b)")
                    nc.vector.copy_predicated(Eedge, masku, w2m[s][:])
                    nc.vector.tensor_tensor(Eedge, Eedge, sgn[s][:], op=ALU.mult)
                    for i, (q, extra) in enumerate(aux):
                        nc.vector.tensor_tensor(
                            E[s][:, AUXB + i:AUXB + i + 1], E[s][:, q:q + 1],
                            E[s][:, extra:extra + 1], op=ALU.add)

                    # ---- colsum: strip gathers + accumulating matmuls ----
                    for h in range(2):
                        o = D_KEEP * HALF * h
                        nc.gpsimd.indirect_copy(
                            G[s][:, h].rearrange("p d c -> p (d c)"),
                            E[s][:, 0:EXTW],
                            sidx[:, o // 16:(o + D_KEEP * HALF) // 16], True)
                    for h in range(2):
                        sl = slice(h * HALF, (h + 1) * HALF)
                        zps = zpsA[s] if h == 0 else zpsB[s]
                        for d in range(D_KEEP):
                            nc.tensor.matmul(zps[:], lhsT=wmat[:],
                                             rhs=G[s][:, h, d, :],
                                             start=(d == 0),
                                             stop=(d == D_KEEP - 1))
                        if it == ITERS - 1:
                            nc.scalar.activation(zout[s][:, sl], zps[:],
                                                 func=AF.Copy)
                            nc.sync.dma_start(out_d[s * B:(s + 1) * B, sl],
                                              zout[s][0:B, sl])
                        else:
                            nc.scalar.activation(zrep[s][:, sl], zps[:],
                                                 func=AF.Copy)

    hoist_waits(nc)
    return nc


# ------------------------------------------------------------ host driver ----
_CACHE = {}


def kernel(r, H, alpha, beta):
    r = np.asarray(r, dtype=np.float32)
    H = np.asarray(H, dtype=np.float32)
    alpha_l = [float(x) for x in np.asarray(alpha).reshape(-1)]
    beta_l = [float(x) for x in np.asarray(beta).reshape(-1)]

    key = (H.tobytes(), tuple(alpha_l), tuple(beta_l))
    if key not in _CACHE:
        tables = build_tables(H)
        nc = build_bass(alpha_l, beta_l, tables["aux"])
        _CACHE[key] = (tables, nc)
    tables, nc = _CACHE[key]

    from concourse.bass_utils import run_bass_kernel_spmd
    in_maps = []
    for c in range(8):
        rs = np.ascontiguousarray(r[c * BC:(c + 1) * BC])
        m = {"zidx": tables["zidx"], "sidx": tables["sidx"],
             "ktab": tables["ktab"], "wmat": tables["wmat"]}
        for s in range(NSTR):
            rstr = np.ascontiguousarray(rs[s * B:(s + 1) * B])
            m[f"x0_{s}"] = build_x0(rstr, tables["colidx"])
            m[f"r4_{s}"] = build_r4(rstr, tables["rcols"])
        in_maps.append(m)
    # the first execution on a freshly-attached device occasionally fails
    # with NRT_EXEC_UNIT_UNRECOVERABLE; a retry succeeds
    last = None
    for _attempt in range(3):
        try:
            res = run_bass_kernel_spmd(nc, in_maps, core_ids=list(range(8)))
            break
        except Exception as e:  # noqa: BLE001
            last = e
    else:
        raise last
    out = np.concatenate([res.results[c]["out"] for c in range(8)], axis=0)
    return out.astype(np.float32)
